# revision 11
# baseline (speedup 1.0000x reference)
"""CausalShapedAttention Trainium2 Bass kernel.

Problem: y = (beta*softmax(causal(q k^T / sqrt(D))) + alpha*I - gamma*MC) @ v
  with qk = x @ w_attn^T (q,k halves), v = x reshaped; B=2, T=2048, C=1024,
  H=16, D=64.  MC[i,j] = 1/(T-1-i) for j>i (i<T-1); MC[T-1,:] = 1/T.

Sharding: 8 cores; core c -> batch b=c//4, head-group g=c%4 (4 heads each),
fully independent (no collectives).  The host passes x^T, the v-slice and the
W-slice^T in fp16 (halves DMA time; validated ~7e-4 rel overall).

Per-core dataflow (natural-layout PV, suffix-sum MC):
  qT,kT = W^T @ xT        fp16 PE matmuls, fp32 PSUM, stored fp32r [64,T]/head
  ST[j,i] = kT_bj . qT    fp32r, 1024-wide query chunks; matmul pieces are
                          kept >=256 cols (fp32r runs 4x slower below that)
                          by extending narrow pieces into dead columns
  exp via ACT (fused 1/sqrt(D) scale) -> fp16 SBUF tiles, one per key block;
                          causal mask = Pool affine_select on diag blocks
  PV in natural layout:   y_bi[i,d] += ex_bj[:,islice]^T @ v_bj, fp16 moving
                          (65 cols/block incl. a 1/beta ones column whose
                          accumulated sum makes reciprocal yield beta/sum);
                          ldweights are free so per-block stationary swaps
                          cost nothing
  MC via suffix sums:     (MC@v)[i,:] = c_i*(sum_{j>i} v_j) + alpha*v_i with
                          c_i = -gamma/(T-1-i); per 128-block: one tri_c
                          matmul (within-block suffix, alpha on nothing),
                          one alpha*I matmul and one rank-1 tail using block
                          suffix sums TF; the dense last row is patched from
                          TF[0] (= colsum of v) over partitions 96:128
  combine per (head,blk): one DVE scalar_tensor_tensor
                          y = yP * (beta/sum)_i + mc_sb, fp16 out, grouped
                          output DMAs

Scheduling: the Tile list-scheduler follows emission priority, so emission
order is shaped for overlap: PE p-state warmup matmuls at t=0, per-cc
interleaved W/x^T input DMAs feeding a cc-major first projection, then ST
chunks woven with filler units (projection quarters, MC blocks, PV units)
sized to each block's exp-stall budget.  PSUM: 3x2-bank ST/projection ring +
2x1-bank ring for PV/MC accumulators.  HW constraints honored: GPSIMD never
touches PSUM, engine APs start at partition 0/32/64/96, no fp32r memsets,
affine_select only uses is_equal/is_ge/is_gt.

Cost-model duration: ~104.1 us/core (baseline 169.6 us).
"""
import sys

for _p in ("/opt/trn_rl_repo",):
    if _p not in sys.path:
        sys.path.insert(0, _p)

from contextlib import ExitStack

import numpy as np

import concourse.bass as bass
import concourse.tile as tile
from concourse import bacc, mybir
from concourse.bass_utils import run_bass_kernel_spmd

F32 = mybir.dt.float32
F32R = mybir.dt.float32r
F16 = mybir.dt.float16
BF16 = mybir.dt.bfloat16
EXP = mybir.ActivationFunctionType.Exp
OP = mybir.AluOpType

B, T, C, H, D = 2, 2048, 1024, 16, 64
HL = 4            # heads per core
GC = HL * D       # channels per head-group (256)
NCORES = 8
NB = T // 128     # 16 key/query row blocks
KC = C // 128     # 8 contraction chunks
CW = 1024         # query-chunk width
NCH = T // CW     # 2 chunks

LAST_RESULTS = None  # BassKernelResults of the most recent run (for test.py)
PHASES = None        # optional [(instr_idx, label)] filled during emission


def _emit(tc: tile.TileContext, xt, xv, wt, y, alpha, beta, gamma):
    nc = tc.nc
    assert beta != 0.0, "beta == 0 unsupported"

    with ExitStack() as ctx:
        ctx.enter_context(nc.allow_low_precision(
            reason="fp16 operands for matmuls; fp16 exp tiles"))
        consts = ctx.enter_context(tc.tile_pool(name="consts", bufs=1))

        # ------- input DMAs: few big transfers (HWDGE slots are ~650ns) -------
        # wqkT[:, cc*512 + mt*128 : +128] = W^T chunk cc, column group mt
        wqkT = consts.tile([128, KC * 512], F16, name="wqkT", tag="wqkT")
        wq3 = wqkT.rearrange("p (cc m) -> p cc m", cc=KC)
        wt3 = wt.rearrange("(cc p) m -> p cc m", p=128)
        # xTa[:, cc*T + t] = x^T[cc*128 + p, t]; interleave the W and x^T
        # streams per contraction chunk so the first projection matmuls
        # unlock as early as possible
        xTa = consts.tile([128, KC * T], F16, name="xTa", tag="xTa")
        xTa3 = xTa.rearrange("p (cc t) -> p cc t", cc=KC)
        xt3 = xt.rearrange("(cc p) t -> p cc t", p=128)
        for c0 in range(0, KC, 2):
            nc.sync.dma_start(out=wq3[:, c0:c0 + 2, :], in_=wt3[:, c0:c0 + 2, :])
            nc.sync.dma_start(out=xTa3[:, c0:c0 + 1, 0:CW],
                              in_=xt3[:, c0:c0 + 1, 0:CW])
            nc.sync.dma_start(out=xTa3[:, c0 + 1:c0 + 2, 0:CW],
                              in_=xt3[:, c0 + 1:c0 + 2, 0:CW])
        for c0 in range(0, KC, 4):
            nc.sync.dma_start(out=xTa3[:, c0:c0 + 4, CW:T],
                              in_=xt3[:, c0:c0 + 4, CW:T])
        # vta[:, bt*264 + m]: m in 0:256 = v block bt, col 256 = 1/beta
        VW = GC + 8
        vta = consts.tile([128, NB * VW], F16, name="vta", tag="vta")
        vta3 = vta.rearrange("p (bt m) -> p bt m", bt=NB)
        nc.sync.dma_start(
            out=vta3[:, :, 0:GC],
            in_=xv.rearrange("(bt p) m -> p bt m", p=128))

        def vt(bt):
            return vta[:, bt * VW:bt * VW + GC + 1]

        # ---------------- constants ----------------
        # PE p-state warmup fodder (never read); memset first so the DVE
        # unblocks the warmup matmuls immediately
        wsrc = consts.tile([128, 128], F32, name="wsrc", tag="wsrc")
        nc.vector.memset(wsrc, 0.0)

        ident = consts.tile([128, 128], F32, name="ident", tag="ident")
        nc.vector.memset(ident, 1.0)
        negmaskT_f = consts.tile([128, 128], F32, name="negmaskT_f",
                                 tag="negmaskT_f")
        nc.vector.memset(negmaskT_f, 0.0)
        nc.gpsimd.affine_select(
            out=negmaskT_f, in_=negmaskT_f, compare_op=OP.is_ge, fill=-1e30,
            base=0, pattern=[[-1, 128]], channel_multiplier=1)
        negmaskT = consts.tile([128, 128], BF16, name="negmaskT",
                               tag="negmaskT")
        nc.vector.tensor_copy(out=negmaskT, in_=negmaskT_f)
        identr = consts.tile([128, 128], BF16, name="identr", tag="identr")
        nc.gpsimd.affine_select(
            out=ident, in_=ident, compare_op=OP.is_equal, fill=0.0,
            base=0, pattern=[[-1, 128]], channel_multiplier=1)
        nc.vector.tensor_copy(out=identr, in_=ident)

        for bt in range(NB):
            nc.vector.memset(vta[:, bt * VW + GC:bt * VW + GC + 1], 1.0 / beta)

        ones1 = consts.tile([1, 128], F16, name="ones1", tag="ones1")
        nc.vector.memset(ones1, 1.0)
        onescol = consts.tile([128, 1], F16, name="onescol", tag="onescol")
        nc.vector.memset(onescol, 1.0)
        aI16 = consts.tile([128, 128], F16, name="aI16", tag="aI16")
        nc.vector.memset(aI16, alpha)
        nc.gpsimd.affine_select(
            out=aI16, in_=aI16, compare_op=OP.is_equal, fill=0.0,
            base=0, pattern=[[-1, 128]], channel_multiplier=1)
        # U01[bj, bi] = 1 if bj >= bi (for TF = suffix-inclusive block sums)
        U01 = consts.tile([16, 16], F16, name="U01", tag="U01")
        nc.vector.memset(U01, 1.0)
        nc.gpsimd.affine_select(
            out=U01, in_=U01, compare_op=OP.is_ge, fill=0.0,
            base=0, pattern=[[-1, 16]], channel_multiplier=1)

        # c_i = -gamma/(T-1-i) as a [1, T] row at partition 0 (rank-1 matmul
        # operands must sit at partition 0 for a legal tile_position)
        c_f32 = consts.tile([1, T], F32, name="c_f32", tag="c_f32")
        nc.gpsimd.iota(c_f32, pattern=[[-1, T]], base=T - 1,
                       channel_multiplier=0,
                       allow_small_or_imprecise_dtypes=True)
        nc.vector.memset(c_f32[0:1, T - 1:T], 1.0)  # avoid 1/0; zeroed below
        nc.vector.reciprocal(out=c_f32, in_=c_f32)
        c_flat = consts.tile([1, T], F16, name="c_flat", tag="c_flat")
        nc.vector.tensor_scalar_mul(c_flat, c_f32, -gamma)
        nc.vector.memset(c_flat[0:1, T - 1:T], 0.0)

        # warm the ACT exp table while DMAs stream
        warm = consts.tile([1, 1], F32, name="warm", tag="warm")
        nc.vector.memset(warm, 0.0)
        nc.scalar.activation(out=warm, in_=warm, func=EXP, scale=1.0)


        # persistent SBUF tensors
        qkT = [consts.tile([128, T], F32R, name=f"qkT{mt}", tag=f"qkT{mt}")
               for mt in range(4)]
        tri_c = [consts.tile([128, 128], F16, name=f"tri{bi}", tag=f"tri{bi}")
                 for bi in range(NB)]
        mc_sb = [consts.tile([128, T], F16, name=f"mcsb{p}", tag=f"mcsb{p}")
                 for p in range(2)]
        TF_flat = [consts.tile([1, T], F16, name=f"tff{p}", tag=f"tff{p}")
                   for p in range(2)]
        tot127 = [consts.tile([128, 128], F16, name=f"tot{p}", tag=f"tot{p}")
                  for p in range(2)]
        ysall = consts.tile([128, NB * GC], F16, name="ysall", tag="ysall")

        def ys(bi):
            return ysall[:, bi * GC:(bi + 1) * GC]

        # ---------------- pools ----------------
        # PSUM: st/proj ring 3x2 banks; everything else shares a 2x1 ring
        stp = ctx.enter_context(tc.tile_pool(name="stp", bufs=3, space="PSUM"))
        ypp = ctx.enter_context(tc.tile_pool(name="ypp", bufs=2, space="PSUM"))
        mcp = ypp
        expool = ctx.enter_context(tc.tile_pool(name="expool", bufs=1))
        mcstage = ctx.enter_context(tc.tile_pool(name="mcstage", bufs=1))
        srp = ctx.enter_context(tc.tile_pool(name="srp", bufs=1))

        exd = {}   # (h, ct, bj) -> ex AP
        ypd = {}   # (h, ct, bi) -> yp AP
        ci = [0]

        def alt_copy(dst, src):
            # PSUM->SBUF evictions live on the DVE: GPSIMD cannot read PSUM
            # (BIR verifier rejects it) and the ACT engine is kept for exp
            nc.vector.tensor_copy(out=dst, in_=src)
            ci[0] += 1

        # ---------------- projection ----------------
        def proj_quarter(nt, mt, half, q, box):
            # 4 contraction chunks; a half-group spans two quarters sharing
            # one PSUM tile (kept in box), copy-out on the second quarter
            if q == 0:
                box[0] = stp.tile([128, CW], F32, name="pp", tag="st")
            pp = box[0]
            x0 = nt * CW + half * 512
            for cc in range(q * 4, q * 4 + 4):
                nc.tensor.matmul(
                    pp[:, 0:512],
                    wqkT[:, cc * 512 + mt * 128:cc * 512 + mt * 128 + 128],
                    xTa[:, cc * T + x0:cc * T + x0 + 512],
                    start=(cc == 0), stop=(cc == KC - 1))
            if q == 1:
                dst = qkT[mt][:, nt * CW + half * 512:
                              nt * CW + half * 512 + 512]
                alt_copy(dst, pp[:, 0:512])

        def proj_units(nt, mt):
            units = []
            for half in range(2):
                box = [None]
                for q in range(2):
                    units.append(
                        (lambda h=half, q=q, b=box:
                         proj_quarter(nt, mt, h, q, b), 860))
            return units

        def proj_header():
            # cc-major emission: all four (mt, half) groups advance as each
            # xTa chunk lands, so the groups complete right after the DMA;
            # copies ride the still-idle ACT engine (no DVE/Pool dependency)
            ppA = stp.tile([128, CW], F32, name="ppA", tag="st")
            ppB = stp.tile([128, CW], F32, name="ppB", tag="st")
            for cc in range(KC):
                for half in range(2):
                    x0 = half * 512
                    for mt, pp in ((0, ppA), (2, ppB)):
                        nc.tensor.matmul(
                            pp[:, x0:x0 + 512],
                            wqkT[:, cc * 512 + mt * 128:
                                 cc * 512 + mt * 128 + 128],
                            xTa[:, cc * T + x0:cc * T + x0 + 512],
                            start=(cc == 0), stop=(cc == KC - 1))
            for half in range(2):
                x0 = half * 512
                nc.scalar.copy(out=qkT[0][:, x0:x0 + 512],
                               in_=ppA[:, x0:x0 + 512])
                nc.vector.tensor_copy(out=qkT[2][:, x0:x0 + 512],
                                      in_=ppB[:, x0:x0 + 512])

        # ---------------- ST + exp per (head, chunk) ----------------
        def exw(bj):
            # allocated ex-tile width for key block bj
            return CW if bj < 8 else CW - (bj - 8) * 128

        def st_block(h, ct, bj):
            qTh = qkT[h // 2][(h % 2) * 64:(h % 2) * 64 + 64, :]
            kTh = qkT[2 + h // 2][(h % 2) * 64:(h % 2) * 64 + 64, :]
            c0 = ct * CW
            if True:
                lo = max(bj * 128, c0)
                rel = lo - c0
                n = CW - rel
                st = stp.tile([128, CW], F32, name="st", tag="st")
                diag = lo == bj * 128
                # matmul pieces within single PSUM banks; pieces narrower than
                # 256 (fp32r runs 4x slower there) are extended leftward into
                # dead space (cols < rel are never read by the exp)
                # the final chunk masks on the PE (inside ST) so the tail
                # critical path skips the Pool affine hop after the last exp
                pe_mask = h == HL - 1 and ct == NCH - 1
                p0 = rel
                while p0 < CW:
                    p1 = min(CW, (p0 // 512) * 512 + 512)
                    q0 = p0 if p1 - p0 >= 256 else p1 - 256
                    masked = pe_mask and diag and p0 == rel
                    nc.tensor.matmul(
                        st[:, q0:p1],
                        kTh[:, bj * 128:(bj + 1) * 128],
                        qTh[:, c0 + q0:c0 + p1],
                        start=True, stop=not masked)
                    if masked:
                        nc.tensor.matmul(
                            st[:, rel:rel + 128], negmaskT, identr,
                            start=False, stop=True)
                    p0 = p1
                ex = expool.tile([128, exw(bj)], F16, name=f"ex{bj}",
                                 tag=f"ex{bj}", bufs=4 if bj < 8 else 2)
                nc.scalar.activation(out=ex[:, 0:n], in_=st[:, rel:CW],
                                     func=EXP, scale=0.125)
                if diag and not pe_mask:
                    # causal mask: zero ex where key j > query i (diag block
                    # occupies the first 128 cols of this ex tile)
                    nc.gpsimd.affine_select(
                        out=ex[:, 0:128], in_=ex[:, 0:128],
                        compare_op=OP.is_ge, fill=0.0,
                        base=0, pattern=[[1, 128]], channel_multiplier=-1)
                exd[(h, ct, bj)] = ex

        # ---------------- PV + combine per (head, chunk) ----------------
        pv_box = {}

        def pv_unit(h, ct, bi, bj_hi=None):
            # bj_hi: emit only blocks < bj_hi now (accumulation stays open in
            # pv_box); a later call finishes the unit and combines.  Lets the
            # final unit's bulk run before the last exp lands.
            c0 = ct * CW
            key = (h, ct, bi)
            if key in pv_box:
                yp, bj0 = pv_box.pop(key)
            else:
                yp = ypp.tile([128, 512], F32, name="yp", tag="yp")
                bj0 = 0
            hi = bi + 1 if bj_hi is None else bj_hi
            for bj in range(bj0, hi):
                lo = max(bj * 128, c0)
                sl = exd[(h, ct, bj)][:, bi * 128 - lo:bi * 128 - lo + 128]
                nc.tensor.matmul(
                    yp[:, 0:64], sl, vt(bj)[:, h * 64:(h + 1) * 64],
                    start=(bj == 0), stop=False, skip_group_check=True)
                nc.tensor.matmul(
                    yp[:, 64:65], sl, vt(bj)[:, GC:GC + 1],
                    start=False, stop=(bj == bi), skip_group_check=True)
            if bj_hi is not None:
                pv_box[key] = (yp, hi)
                return
            emit_combine(h, ct, bi, yp)

        def pv_units(h, ct):
            return [(lambda bi=bi: pv_unit(h, ct, bi), 30 * (bi + 1) + 120)
                    for bi in range(ct * 8, ct * 8 + 8)]

        def emit_combine(h, ct, bi, yp):
            rc = srp.tile([128, 1], F32, name="rc", tag="rc", bufs=4)
            nc.vector.reciprocal(out=rc, in_=yp[:, 64:65])
            p = h // 2
            mcsl = mc_sb[p][:, bi * 128 + (h % 2) * 64:
                            bi * 128 + (h % 2) * 64 + 64]
            nc.vector.scalar_tensor_tensor(
                out=ys(bi)[:, h * 64:(h + 1) * 64],
                in0=yp[:, 0:64], scalar=rc, in1=mcsl,
                op0=OP.mult, op1=OP.add)
            if bi == NB - 1 and h % 2 == 1:
                # dense last row of MC: y[T-1,:] += -gamma/T * colsum(v);
                # colsum = TF[0], DMA'd to partition 127 of tot127 (rest
                # zeroed).  Engine APs must start at partition 0/32/64/96,
                # so run over [96:128] - rows 96..126 add 0*colsum = 0.
                # Emitted per pair right after that pair's last combine.
                q = h // 2
                sl2 = ys(bi)[96:128, q * 128:(q + 1) * 128]
                nc.vector.scalar_tensor_tensor(
                    out=sl2, in0=tot127[q][96:128, 0:128],
                    scalar=-gamma / T, in1=sl2,
                    op0=OP.mult, op1=OP.add)
            if h == HL - 1:
                # grouped output DMAs (HWDGE slots cost ~650ns each)
                groups = {7: (0, 8), 11: (8, 12), 13: (12, 14),
                          14: (14, 15), 15: (15, 16)}
                if bi in groups:
                    a, b = groups[bi]
                    nc.sync.dma_start(
                        out=y.rearrange("(b p) m -> p b m", p=128)[:, a:b, :],
                        in_=ysall.rearrange("p (b m) -> p b m", b=NB)[:, a:b, :])

        # ---------------- MC machinery ----------------
        def emit_mc_setup(p):
            # block column sums bs2[chan, bj], suffix-inclusive TF[bi, chan],
            # flattened to [1, T] for rank-1 tail matmuls
            ch = slice(p * 128, (p + 1) * 128)
            bs2 = ypp.tile([128, 512], F32, name="bs2", tag="yp")
            for bj in range(NB):
                nc.tensor.matmul(bs2[:, bj:bj + 1], vt(bj)[:, ch], onescol,
                                 start=(bj == 0), stop=(bj == NB - 1),
                                 skip_group_check=True)
            bs2sb = mcstage.tile([128, 16], F32, name=f"bs2sb{p}", tag="bs2sb",
                                 bufs=2)
            nc.vector.tensor_copy(out=bs2sb, in_=bs2[:, 0:16])
            bs2T = ypp.tile([128, 512], F32, name="bs2T", tag="yp")
            nc.tensor.transpose(bs2T[0:16, 0:128], bs2sb, ident)
            bs2T16 = mcstage.tile([16, 128], F16, name=f"bs2T16{p}",
                                  tag="bs2T16", bufs=2)
            nc.vector.tensor_copy(out=bs2T16, in_=bs2T[0:16, 0:128])
            tf = ypp.tile([128, 512], F32, name="tf", tag="yp")
            nc.tensor.matmul(tf[0:16, 0:128], U01, bs2T16, start=True,
                             stop=True)
            tf16 = mcstage.tile([16, 128], F16, name=f"tf16{p}", tag="tf16",
                                bufs=2)
            nc.vector.tensor_copy(out=tf16, in_=tf[0:16, 0:128])
            # flatten [16,128] -> [1,2048]: DMA traversal orders match (p-major)
            nc.sync.dma_start(out=TF_flat[p], in_=tf16)
            nc.vector.memset(tot127[p][96:128, 0:128], 0.0)
            nc.sync.dma_start(out=tot127[p][127:128, 0:128],
                              in_=TF_flat[p][0:1, 0:128])

        def tri_unit(b0):
            for bi in range(b0, b0 + 4):
                trip = mcp.tile([128, 512], F32, name="trip", tag="yp")
                nc.tensor.matmul(trip[:, 0:128], ones1,
                                 c_flat[0:1, bi * 128:(bi + 1) * 128],
                                 start=True, stop=True)
                nc.vector.tensor_copy(out=tri_c[bi], in_=trip[:, 0:128])
                nc.gpsimd.affine_select(
                    out=tri_c[bi], in_=tri_c[bi], compare_op=OP.is_gt,
                    fill=0.0, base=0, pattern=[[-1, 128]],
                    channel_multiplier=1)

        def mcb_unit(p, bi):
            ch = slice(p * 128, (p + 1) * 128)
            if True:
                mcps = mcp.tile([128, 512], F32, name="mcps", tag="yp")
                nc.tensor.matmul(mcps[:, 0:128], tri_c[bi], vt(bi)[:, ch],
                                 start=True, stop=False, skip_group_check=True)
                nc.tensor.matmul(mcps[:, 0:128], aI16, vt(bi)[:, ch],
                                 start=False, stop=(bi == NB - 1),
                                 skip_group_check=True)
                if bi < NB - 1:
                    nc.tensor.matmul(
                        mcps[:, 0:128], c_flat[0:1, bi * 128:(bi + 1) * 128],
                        TF_flat[p][0:1, (bi + 1) * 128:(bi + 2) * 128],
                        start=False, stop=True, skip_group_check=True)
                alt_copy(mc_sb[p][:, bi * 128:(bi + 1) * 128], mcps[:, 0:128])

        # ---------------- schedule ----------------
        def mark(lab):
            if PHASES is not None:
                idx = int(nc.get_next_instruction_name().split('-')[1])
                PHASES.append((idx, lab))

        # PE p-state warmup: throwaway matmuls start the 3us ramp clock at
        # t~0 so the first DMA-gated projections run at full clock
        for wi in range(14):
            wps = ypp.tile([128, 512], F32, name="wps", tag="yp")
            nc.tensor.matmul(wps[:, 0:128], wsrc.bitcast(F32R),
                             wsrc.bitcast(F32R), start=True, stop=True,
                             skip_group_check=True)

        mark('header')
        proj_header()
        mark('header_end')

        def weave(chunk, fillers):
            # interleave filler units into the ST chunk's exp-paced stalls,
            # weighted by each block's stall budget (exp time - ST time) so
            # the short trailing blocks are not clogged by filler work
            h, ct = chunk
            nblk = ct * 8 + 8
            costs = [f[1] for f in fillers]
            total = sum(costs) or 1
            budgets = []
            c0 = ct * CW
            for bj in range(nblk):
                n = CW - (max(bj * 128, c0) - c0)
                budgets.append(max((n + 222) / 1.2 - n / 2.4, 0.0))
            btot = sum(budgets)
            done = 0
            acc = 0.0
            spent = 0.0
            for bj in range(nblk):
                mark(f"st{chunk}b{bj}")
                st_block(h, ct, bj)
                acc += budgets[bj]
                while done < len(fillers) and (
                        spent + costs[done] <= acc * total / max(btot, 1)
                        or bj == nblk - 1):
                    mark(f"fill{chunk}#{done}")
                    fillers[done][0]()
                    spent += costs[done]
                    done += 1

        plan = [
            ((0, 0), proj_units(1, 0)),
            ((1, 0), proj_units(1, 2)
                     + [(lambda: emit_mc_setup(0), 700),
                        (lambda: emit_mc_setup(1), 700)]),
            ((0, 1), [(lambda b=b: tri_unit(b), 350) for b in (0, 4, 8, 12)]
                     + [(lambda bi=bi: mcb_unit(0, bi), 220)
                        for bi in range(NB)]
                     + pv_units(0, 0) + proj_units(0, 1)[:2]),
            ((1, 1), pv_units(1, 0) + proj_units(0, 1)[2:]
                     + proj_units(0, 3)),
            ((2, 0), proj_units(1, 1)),
            ((3, 0), pv_units(0, 1)),
            ((2, 1), proj_units(1, 3)
                     + [(lambda bi=bi: mcb_unit(1, bi), 220)
                        for bi in range(NB)]),
            ((3, 1), pv_units(1, 1) + pv_units(2, 0) + pv_units(3, 0) + pv_units(2, 1)
                     + pv_units(3, 1)[:6]
                     + [(lambda: pv_unit(3, 1, 14, bj_hi=13), 400),
                        (lambda: pv_unit(3, 1, 15, bj_hi=14), 430)]),
        ]
        for chunk, fillers in plan:
            weave(chunk, fillers)
        pv_unit(3, 1, 14)
        pv_unit(3, 1, 15)


_BUILD_CACHE = {}


def build_nc(alpha, beta, gamma):
    key = (alpha, beta, gamma)
    if key in _BUILD_CACHE:
        return _BUILD_CACHE[key]
    nc = bacc.Bacc("TRN2", target_bir_lowering=False, debug=False,
                   num_devices=NCORES)
    xt = nc.dram_tensor("xt", [C, T], F16, kind="ExternalInput").ap()
    xv = nc.dram_tensor("xv", [T, GC], F16, kind="ExternalInput").ap()
    wt = nc.dram_tensor("wt", [C, 2 * GC], F16, kind="ExternalInput").ap()
    y = nc.dram_tensor("y", [T, GC], F16, kind="ExternalOutput").ap()
    with tile.TileContext(nc) as tc:
        _emit(tc, xt, xv, wt, y, alpha, beta, gamma)
    nc.compile()
    _BUILD_CACHE[key] = nc
    return nc


def make_in_maps(x, w):
    xts = [np.ascontiguousarray(x[b].T).astype(np.float16) for b in range(B)]
    in_maps = []
    for c in range(NCORES):
        b, g = c // HL, c % HL
        wqk = np.concatenate(
            [w[GC * g:GC * (g + 1)], w[C + GC * g:C + GC * (g + 1)]], axis=0)
        in_maps.append({
            "xt": xts[b],
            "xv": np.ascontiguousarray(
                x[b][:, GC * g:GC * (g + 1)]).astype(np.float16),
            "wt": np.ascontiguousarray(wqk.T).astype(np.float16),
        })
    return in_maps


def kernel(x, w_attn, alpha, beta, gamma, n_head, **run_kwargs):
    global LAST_RESULTS
    x = np.asarray(x, dtype=np.float32)
    w = np.asarray(w_attn, dtype=np.float32)
    assert int(n_head) == H and x.shape == (B, T, C)
    nc = build_nc(float(alpha), float(beta), float(gamma))
    res = run_bass_kernel_spmd(nc, make_in_maps(x, w), list(range(NCORES)),
                               **run_kwargs)
    LAST_RESULTS = res
    out = np.empty((B, T, C), dtype=np.float32)
    for c in range(NCORES):
        b, g = c // HL, c % HL
        out[b][:, GC * g:GC * (g + 1)] = res.results[c]["y"].astype(np.float32)
    return out


# revision 12
# speedup vs baseline: 1.0021x; 1.0021x over previous
"""CausalShapedAttention Trainium2 Bass kernel.

Problem: y = (beta*softmax(causal(q k^T / sqrt(D))) + alpha*I - gamma*MC) @ v
  with qk = x @ w_attn^T (q,k halves), v = x reshaped; B=2, T=2048, C=1024,
  H=16, D=64.  MC[i,j] = 1/(T-1-i) for j>i (i<T-1); MC[T-1,:] = 1/T.

Sharding: 8 cores; core c -> batch b=c//4, head-group g=c%4 (4 heads each),
fully independent (no collectives).  The host passes x^T, the v-slice and the
W-slice^T in fp16 (halves DMA time; validated ~7e-4 rel overall).

Per-core dataflow (natural-layout PV, suffix-sum MC):
  qT,kT = W^T @ xT        fp16 PE matmuls, fp32 PSUM, stored fp32r [64,T]/head
  ST[j,i] = kT_bj . qT    fp32r, 1024-wide query chunks; matmul pieces are
                          kept >=256 cols (fp32r runs 4x slower below that)
                          by extending narrow pieces into dead columns
  exp via ACT (fused 1/sqrt(D) scale) -> fp16 SBUF tiles, one per key block;
                          causal mask = Pool affine_select on diag blocks
  PV in natural layout:   y_bi[i,d] += ex_bj[:,islice]^T @ v_bj, fp16 moving
                          (65 cols/block incl. a 1/beta ones column whose
                          accumulated sum makes reciprocal yield beta/sum);
                          ldweights are free so per-block stationary swaps
                          cost nothing
  MC via suffix sums:     (MC@v)[i,:] = c_i*(sum_{j>i} v_j) + alpha*v_i with
                          c_i = -gamma/(T-1-i); per 128-block: one tri_c
                          matmul (within-block suffix, alpha on nothing),
                          one alpha*I matmul and one rank-1 tail using block
                          suffix sums TF; the dense last row is patched from
                          TF[0] (= colsum of v) over partitions 96:128
  combine per (head,blk): one DVE scalar_tensor_tensor
                          y = yP * (beta/sum)_i + mc_sb, fp16 out, grouped
                          output DMAs

Scheduling: the Tile list-scheduler follows emission priority, so emission
order is shaped for overlap: PE p-state warmup matmuls at t=0, per-cc
interleaved W/x^T input DMAs feeding a cc-major first projection, then ST
chunks woven with filler units (projection quarters, MC blocks, PV units)
sized to each block's exp-stall budget.  PSUM: 3x2-bank ST/projection ring +
2x1-bank ring for PV/MC accumulators.  HW constraints honored: GPSIMD never
touches PSUM, engine APs start at partition 0/32/64/96, no fp32r memsets,
affine_select only uses is_equal/is_ge/is_gt.

Cost-model duration: ~104.1 us/core (baseline 169.6 us).
"""
import sys

for _p in ("/opt/trn_rl_repo",):
    if _p not in sys.path:
        sys.path.insert(0, _p)

from contextlib import ExitStack

import numpy as np

import concourse.bass as bass
import concourse.tile as tile
from concourse import bacc, mybir
from concourse.bass_utils import run_bass_kernel_spmd

F32 = mybir.dt.float32
F32R = mybir.dt.float32r
F16 = mybir.dt.float16
BF16 = mybir.dt.bfloat16
EXP = mybir.ActivationFunctionType.Exp
OP = mybir.AluOpType

B, T, C, H, D = 2, 2048, 1024, 16, 64
HL = 4            # heads per core
GC = HL * D       # channels per head-group (256)
NCORES = 8
NB = T // 128     # 16 key/query row blocks
KC = C // 128     # 8 contraction chunks
CW = 1024         # query-chunk width
NCH = T // CW     # 2 chunks

LAST_RESULTS = None  # BassKernelResults of the most recent run (for test.py)
PHASES = None        # optional [(instr_idx, label)] filled during emission


def _emit(tc: tile.TileContext, xt, xv, wt, y, alpha, beta, gamma):
    nc = tc.nc
    assert beta != 0.0, "beta == 0 unsupported"

    with ExitStack() as ctx:
        ctx.enter_context(nc.allow_low_precision(
            reason="fp16 operands for matmuls; fp16 exp tiles"))
        consts = ctx.enter_context(tc.tile_pool(name="consts", bufs=1))

        # ------- input DMAs: few big transfers (HWDGE slots are ~650ns) -------
        # wqkT[:, cc*512 + mt*128 : +128] = W^T chunk cc, column group mt
        wqkT = consts.tile([128, KC * 512], F16, name="wqkT", tag="wqkT")
        wq3 = wqkT.rearrange("p (cc m) -> p cc m", cc=KC)
        wt3 = wt.rearrange("(cc p) m -> p cc m", p=128)
        # xTa[:, cc*T + t] = x^T[cc*128 + p, t]; interleave the W and x^T
        # streams per contraction chunk so the first projection matmuls
        # unlock as early as possible
        xTa = consts.tile([128, KC * T], F16, name="xTa", tag="xTa")
        xTa3 = xTa.rearrange("p (cc t) -> p cc t", cc=KC)
        xt3 = xt.rearrange("(cc p) t -> p cc t", p=128)
        for c0 in range(0, KC, 2):
            nc.sync.dma_start(out=wq3[:, c0:c0 + 2, :], in_=wt3[:, c0:c0 + 2, :])
            nc.sync.dma_start(out=xTa3[:, c0:c0 + 1, 0:CW],
                              in_=xt3[:, c0:c0 + 1, 0:CW])
            nc.sync.dma_start(out=xTa3[:, c0 + 1:c0 + 2, 0:CW],
                              in_=xt3[:, c0 + 1:c0 + 2, 0:CW])
        for c0 in range(0, KC, 4):
            nc.sync.dma_start(out=xTa3[:, c0:c0 + 4, CW:T],
                              in_=xt3[:, c0:c0 + 4, CW:T])
        # vta[:, bt*264 + m]: m in 0:256 = v block bt, col 256 = 1/beta
        VW = GC + 8
        vta = consts.tile([128, NB * VW], F16, name="vta", tag="vta")
        vta3 = vta.rearrange("p (bt m) -> p bt m", bt=NB)
        nc.sync.dma_start(
            out=vta3[:, :, 0:GC],
            in_=xv.rearrange("(bt p) m -> p bt m", p=128))

        def vt(bt):
            return vta[:, bt * VW:bt * VW + GC + 1]

        # ---------------- constants ----------------
        # PE p-state warmup fodder (never read); memset first so the DVE
        # unblocks the warmup matmuls immediately
        wsrc = consts.tile([128, 128], F32, name="wsrc", tag="wsrc")
        nc.vector.memset(wsrc, 0.0)

        ident = consts.tile([128, 128], F32, name="ident", tag="ident")
        nc.vector.memset(ident, 1.0)
        negmaskT_f = consts.tile([128, 128], F32, name="negmaskT_f",
                                 tag="negmaskT_f")
        nc.vector.memset(negmaskT_f, 0.0)
        nc.gpsimd.affine_select(
            out=negmaskT_f, in_=negmaskT_f, compare_op=OP.is_ge, fill=-1e30,
            base=0, pattern=[[-1, 128]], channel_multiplier=1)
        negmaskT = consts.tile([128, 128], BF16, name="negmaskT",
                               tag="negmaskT")
        nc.vector.tensor_copy(out=negmaskT, in_=negmaskT_f)
        identr = consts.tile([128, 128], BF16, name="identr", tag="identr")
        nc.gpsimd.affine_select(
            out=ident, in_=ident, compare_op=OP.is_equal, fill=0.0,
            base=0, pattern=[[-1, 128]], channel_multiplier=1)
        nc.vector.tensor_copy(out=identr, in_=ident)

        for bt in range(NB):
            nc.vector.memset(vta[:, bt * VW + GC:bt * VW + GC + 1], 1.0 / beta)

        ones1 = consts.tile([1, 128], F16, name="ones1", tag="ones1")
        nc.vector.memset(ones1, 1.0)
        onescol = consts.tile([128, 1], F16, name="onescol", tag="onescol")
        nc.vector.memset(onescol, 1.0)
        aI16 = consts.tile([128, 128], F16, name="aI16", tag="aI16")
        nc.vector.memset(aI16, alpha)
        nc.gpsimd.affine_select(
            out=aI16, in_=aI16, compare_op=OP.is_equal, fill=0.0,
            base=0, pattern=[[-1, 128]], channel_multiplier=1)
        # U01[bj, bi] = 1 if bj >= bi (for TF = suffix-inclusive block sums)
        U01 = consts.tile([16, 16], F16, name="U01", tag="U01")
        nc.vector.memset(U01, 1.0)
        nc.gpsimd.affine_select(
            out=U01, in_=U01, compare_op=OP.is_ge, fill=0.0,
            base=0, pattern=[[-1, 16]], channel_multiplier=1)

        # c_i = -gamma/(T-1-i) as a [1, T] row at partition 0 (rank-1 matmul
        # operands must sit at partition 0 for a legal tile_position)
        c_f32 = consts.tile([1, T], F32, name="c_f32", tag="c_f32")
        nc.gpsimd.iota(c_f32, pattern=[[-1, T]], base=T - 1,
                       channel_multiplier=0,
                       allow_small_or_imprecise_dtypes=True)
        nc.vector.memset(c_f32[0:1, T - 1:T], 1.0)  # avoid 1/0; zeroed below
        nc.vector.reciprocal(out=c_f32, in_=c_f32)
        c_flat = consts.tile([1, T], F16, name="c_flat", tag="c_flat")
        nc.vector.tensor_scalar_mul(c_flat, c_f32, -gamma)
        nc.vector.memset(c_flat[0:1, T - 1:T], 0.0)

        # warm the ACT exp table while DMAs stream
        warm = consts.tile([1, 1], F32, name="warm", tag="warm")
        nc.vector.memset(warm, 0.0)
        nc.scalar.activation(out=warm, in_=warm, func=EXP, scale=1.0)


        # persistent SBUF tensors
        qkT = [consts.tile([128, T], F32R, name=f"qkT{mt}", tag=f"qkT{mt}")
               for mt in range(4)]
        tri_c = [consts.tile([128, 128], F16, name=f"tri{bi}", tag=f"tri{bi}")
                 for bi in range(NB)]
        mc_sb = [consts.tile([128, T], F16, name=f"mcsb{p}", tag=f"mcsb{p}")
                 for p in range(2)]
        TF_flat = [consts.tile([1, T], F16, name=f"tff{p}", tag=f"tff{p}")
                   for p in range(2)]
        tot127 = [consts.tile([128, 128], F16, name=f"tot{p}", tag=f"tot{p}")
                  for p in range(2)]
        ysall = consts.tile([128, NB * GC], F16, name="ysall", tag="ysall")

        def ys(bi):
            return ysall[:, bi * GC:(bi + 1) * GC]

        # ---------------- pools ----------------
        # PSUM: st/proj ring 3x2 banks; everything else shares a 2x1 ring
        stp = ctx.enter_context(tc.tile_pool(name="stp", bufs=3, space="PSUM"))
        ypp = ctx.enter_context(tc.tile_pool(name="ypp", bufs=2, space="PSUM"))
        mcp = ypp
        expool = ctx.enter_context(tc.tile_pool(name="expool", bufs=1))
        mcstage = ctx.enter_context(tc.tile_pool(name="mcstage", bufs=1))
        srp = ctx.enter_context(tc.tile_pool(name="srp", bufs=1))

        exd = {}   # (h, ct, bj) -> ex AP
        ypd = {}   # (h, ct, bi) -> yp AP
        ci = [0]

        def alt_copy(dst, src):
            # PSUM->SBUF evictions live on the DVE: GPSIMD cannot read PSUM
            # (BIR verifier rejects it) and the ACT engine is kept for exp
            nc.vector.tensor_copy(out=dst, in_=src)
            ci[0] += 1

        # ---------------- projection ----------------
        def proj_quarter(nt, mt, half, q, box):
            # 4 contraction chunks; a half-group spans two quarters sharing
            # one PSUM tile (kept in box), copy-out on the second quarter
            if q == 0:
                box[0] = stp.tile([128, CW], F32, name="pp", tag="st")
            pp = box[0]
            x0 = nt * CW + half * 512
            for cc in range(q * 4, q * 4 + 4):
                nc.tensor.matmul(
                    pp[:, 0:512],
                    wqkT[:, cc * 512 + mt * 128:cc * 512 + mt * 128 + 128],
                    xTa[:, cc * T + x0:cc * T + x0 + 512],
                    start=(cc == 0), stop=(cc == KC - 1))
            if q == 1:
                dst = qkT[mt][:, nt * CW + half * 512:
                              nt * CW + half * 512 + 512]
                alt_copy(dst, pp[:, 0:512])

        def proj_units(nt, mt):
            units = []
            for half in range(2):
                box = [None]
                for q in range(2):
                    units.append(
                        (lambda h=half, q=q, b=box:
                         proj_quarter(nt, mt, h, q, b), 860))
            return units

        def proj_header():
            # cc-major emission: all four (mt, half) groups advance as each
            # xTa chunk lands, so the groups complete right after the DMA;
            # copies ride the still-idle ACT engine (no DVE/Pool dependency)
            ppA = stp.tile([128, CW], F32, name="ppA", tag="st")
            ppB = stp.tile([128, CW], F32, name="ppB", tag="st")
            for cc in range(KC):
                for half in range(2):
                    x0 = half * 512
                    for mt, pp in ((0, ppA), (2, ppB)):
                        nc.tensor.matmul(
                            pp[:, x0:x0 + 512],
                            wqkT[:, cc * 512 + mt * 128:
                                 cc * 512 + mt * 128 + 128],
                            xTa[:, cc * T + x0:cc * T + x0 + 512],
                            start=(cc == 0), stop=(cc == KC - 1))
            for half in range(2):
                x0 = half * 512
                nc.scalar.copy(out=qkT[0][:, x0:x0 + 512],
                               in_=ppA[:, x0:x0 + 512])
                nc.vector.tensor_copy(out=qkT[2][:, x0:x0 + 512],
                                      in_=ppB[:, x0:x0 + 512])

        # ---------------- ST + exp per (head, chunk) ----------------
        def exw(bj):
            # allocated ex-tile width for key block bj
            return CW if bj < 8 else CW - (bj - 8) * 128

        def st_block(h, ct, bj):
            qTh = qkT[h // 2][(h % 2) * 64:(h % 2) * 64 + 64, :]
            kTh = qkT[2 + h // 2][(h % 2) * 64:(h % 2) * 64 + 64, :]
            c0 = ct * CW
            if True:
                lo = max(bj * 128, c0)
                rel = lo - c0
                n = CW - rel
                st = stp.tile([128, CW], F32, name="st", tag="st")
                diag = lo == bj * 128
                # matmul pieces within single PSUM banks; pieces narrower than
                # 256 (fp32r runs 4x slower there) are extended leftward into
                # dead space (cols < rel are never read by the exp)
                # the final chunk masks on the PE (inside ST) so the tail
                # critical path skips the Pool affine hop after the last exp
                pe_mask = h == HL - 1 and ct == NCH - 1
                p0 = rel
                while p0 < CW:
                    p1 = min(CW, (p0 // 512) * 512 + 512)
                    q0 = p0 if p1 - p0 >= 256 else p1 - 256
                    masked = pe_mask and diag and p0 == rel
                    nc.tensor.matmul(
                        st[:, q0:p1],
                        kTh[:, bj * 128:(bj + 1) * 128],
                        qTh[:, c0 + q0:c0 + p1],
                        start=True, stop=not masked)
                    if masked:
                        nc.tensor.matmul(
                            st[:, rel:rel + 128], negmaskT, identr,
                            start=False, stop=True)
                    p0 = p1
                ex = expool.tile([128, exw(bj)], F16, name=f"ex{bj}",
                                 tag=f"ex{bj}", bufs=4 if bj < 8 else 2)
                if (h, ct, bj) == (0, 0, 0):
                    # startup: split the first exp per PSUM bank so the ACT
                    # stream begins as soon as the first ST piece lands
                    nc.scalar.activation(out=ex[:, 0:512], in_=st[:, 0:512],
                                         func=EXP, scale=0.125)
                    nc.scalar.activation(out=ex[:, 512:CW], in_=st[:, 512:CW],
                                         func=EXP, scale=0.125)
                else:
                    nc.scalar.activation(out=ex[:, 0:n], in_=st[:, rel:CW],
                                         func=EXP, scale=0.125)
                if diag and not pe_mask:
                    # causal mask: zero ex where key j > query i (diag block
                    # occupies the first 128 cols of this ex tile)
                    nc.gpsimd.affine_select(
                        out=ex[:, 0:128], in_=ex[:, 0:128],
                        compare_op=OP.is_ge, fill=0.0,
                        base=0, pattern=[[1, 128]], channel_multiplier=-1)
                exd[(h, ct, bj)] = ex

        # ---------------- PV + combine per (head, chunk) ----------------
        pv_box = {}

        def pv_unit(h, ct, bi, bj_hi=None):
            # bj_hi: emit only blocks < bj_hi now (accumulation stays open in
            # pv_box); a later call finishes the unit and combines.  Lets the
            # final unit's bulk run before the last exp lands.
            c0 = ct * CW
            key = (h, ct, bi)
            if key in pv_box:
                yp, bj0 = pv_box.pop(key)
            else:
                yp = ypp.tile([128, 512], F32, name="yp", tag="yp")
                bj0 = 0
            hi = bi + 1 if bj_hi is None else bj_hi
            for bj in range(bj0, hi):
                lo = max(bj * 128, c0)
                sl = exd[(h, ct, bj)][:, bi * 128 - lo:bi * 128 - lo + 128]
                nc.tensor.matmul(
                    yp[:, 0:64], sl, vt(bj)[:, h * 64:(h + 1) * 64],
                    start=(bj == 0), stop=False, skip_group_check=True)
                nc.tensor.matmul(
                    yp[:, 64:65], sl, vt(bj)[:, GC:GC + 1],
                    start=False, stop=(bj == bi), skip_group_check=True)
            if bj_hi is not None:
                pv_box[key] = (yp, hi)
                return
            emit_combine(h, ct, bi, yp)

        def pv_units(h, ct):
            return [(lambda bi=bi: pv_unit(h, ct, bi), 30 * (bi + 1) + 120)
                    for bi in range(ct * 8, ct * 8 + 8)]

        def emit_combine(h, ct, bi, yp):
            rc = srp.tile([128, 1], F32, name="rc", tag="rc", bufs=4)
            nc.vector.reciprocal(out=rc, in_=yp[:, 64:65])
            p = h // 2
            mcsl = mc_sb[p][:, bi * 128 + (h % 2) * 64:
                            bi * 128 + (h % 2) * 64 + 64]
            nc.vector.scalar_tensor_tensor(
                out=ys(bi)[:, h * 64:(h + 1) * 64],
                in0=yp[:, 0:64], scalar=rc, in1=mcsl,
                op0=OP.mult, op1=OP.add)
            if bi == NB - 1 and h % 2 == 1:
                # dense last row of MC: y[T-1,:] += -gamma/T * colsum(v);
                # colsum = TF[0], DMA'd to partition 127 of tot127 (rest
                # zeroed).  Engine APs must start at partition 0/32/64/96,
                # so run over [96:128] - rows 96..126 add 0*colsum = 0.
                # Emitted per pair right after that pair's last combine.
                q = h // 2
                sl2 = ys(bi)[96:128, q * 128:(q + 1) * 128]
                nc.vector.scalar_tensor_tensor(
                    out=sl2, in0=tot127[q][96:128, 0:128],
                    scalar=-gamma / T, in1=sl2,
                    op0=OP.mult, op1=OP.add)
            if h == HL - 1:
                # grouped output DMAs (HWDGE slots cost ~650ns each)
                groups = {7: (0, 8), 11: (8, 12), 13: (12, 14),
                          14: (14, 15), 15: (15, 16)}
                if bi in groups:
                    a, b = groups[bi]
                    nc.sync.dma_start(
                        out=y.rearrange("(b p) m -> p b m", p=128)[:, a:b, :],
                        in_=ysall.rearrange("p (b m) -> p b m", b=NB)[:, a:b, :])

        # ---------------- MC machinery ----------------
        def emit_mc_setup(p):
            # block column sums bs2[chan, bj], suffix-inclusive TF[bi, chan],
            # flattened to [1, T] for rank-1 tail matmuls
            ch = slice(p * 128, (p + 1) * 128)
            bs2 = ypp.tile([128, 512], F32, name="bs2", tag="yp")
            for bj in range(NB):
                nc.tensor.matmul(bs2[:, bj:bj + 1], vt(bj)[:, ch], onescol,
                                 start=(bj == 0), stop=(bj == NB - 1),
                                 skip_group_check=True)
            bs2sb = mcstage.tile([128, 16], F32, name=f"bs2sb{p}", tag="bs2sb",
                                 bufs=2)
            nc.vector.tensor_copy(out=bs2sb, in_=bs2[:, 0:16])
            bs2T = ypp.tile([128, 512], F32, name="bs2T", tag="yp")
            nc.tensor.transpose(bs2T[0:16, 0:128], bs2sb, ident)
            bs2T16 = mcstage.tile([16, 128], F16, name=f"bs2T16{p}",
                                  tag="bs2T16", bufs=2)
            nc.vector.tensor_copy(out=bs2T16, in_=bs2T[0:16, 0:128])
            tf = ypp.tile([128, 512], F32, name="tf", tag="yp")
            nc.tensor.matmul(tf[0:16, 0:128], U01, bs2T16, start=True,
                             stop=True)
            tf16 = mcstage.tile([16, 128], F16, name=f"tf16{p}", tag="tf16",
                                bufs=2)
            nc.vector.tensor_copy(out=tf16, in_=tf[0:16, 0:128])
            # flatten [16,128] -> [1,2048]: DMA traversal orders match (p-major)
            nc.sync.dma_start(out=TF_flat[p], in_=tf16)
            nc.vector.memset(tot127[p][96:128, 0:128], 0.0)
            nc.sync.dma_start(out=tot127[p][127:128, 0:128],
                              in_=TF_flat[p][0:1, 0:128])

        def tri_unit(b0):
            for bi in range(b0, b0 + 4):
                trip = mcp.tile([128, 512], F32, name="trip", tag="yp")
                nc.tensor.matmul(trip[:, 0:128], ones1,
                                 c_flat[0:1, bi * 128:(bi + 1) * 128],
                                 start=True, stop=True)
                nc.vector.tensor_copy(out=tri_c[bi], in_=trip[:, 0:128])
                nc.gpsimd.affine_select(
                    out=tri_c[bi], in_=tri_c[bi], compare_op=OP.is_gt,
                    fill=0.0, base=0, pattern=[[-1, 128]],
                    channel_multiplier=1)

        def mcb_unit(p, bi):
            ch = slice(p * 128, (p + 1) * 128)
            if True:
                mcps = mcp.tile([128, 512], F32, name="mcps", tag="yp")
                nc.tensor.matmul(mcps[:, 0:128], tri_c[bi], vt(bi)[:, ch],
                                 start=True, stop=False, skip_group_check=True)
                nc.tensor.matmul(mcps[:, 0:128], aI16, vt(bi)[:, ch],
                                 start=False, stop=(bi == NB - 1),
                                 skip_group_check=True)
                if bi < NB - 1:
                    nc.tensor.matmul(
                        mcps[:, 0:128], c_flat[0:1, bi * 128:(bi + 1) * 128],
                        TF_flat[p][0:1, (bi + 1) * 128:(bi + 2) * 128],
                        start=False, stop=True, skip_group_check=True)
                alt_copy(mc_sb[p][:, bi * 128:(bi + 1) * 128], mcps[:, 0:128])

        # ---------------- schedule ----------------
        def mark(lab):
            if PHASES is not None:
                idx = int(nc.get_next_instruction_name().split('-')[1])
                PHASES.append((idx, lab))

        # PE p-state warmup: throwaway matmuls start the 3us ramp clock at
        # t~0 so the first DMA-gated projections run at full clock
        for wi in range(14):
            wps = ypp.tile([128, 512], F32, name="wps", tag="yp")
            nc.tensor.matmul(wps[:, 0:128], wsrc.bitcast(F32R),
                             wsrc.bitcast(F32R), start=True, stop=True,
                             skip_group_check=True)

        mark('header')
        proj_header()
        mark('header_end')

        def weave(chunk, fillers):
            # interleave filler units into the ST chunk's exp-paced stalls,
            # weighted by each block's stall budget (exp time - ST time) so
            # the short trailing blocks are not clogged by filler work
            h, ct = chunk
            nblk = ct * 8 + 8
            costs = [f[1] for f in fillers]
            total = sum(costs) or 1
            budgets = []
            c0 = ct * CW
            for bj in range(nblk):
                n = CW - (max(bj * 128, c0) - c0)
                budgets.append(max((n + 222) / 1.2 - n / 2.4, 0.0))
            btot = sum(budgets)
            done = 0
            acc = 0.0
            spent = 0.0
            for bj in range(nblk):
                mark(f"st{chunk}b{bj}")
                st_block(h, ct, bj)
                acc += budgets[bj]
                while done < len(fillers) and (
                        spent + costs[done] <= acc * total / max(btot, 1)
                        or bj == nblk - 1):
                    mark(f"fill{chunk}#{done}")
                    fillers[done][0]()
                    spent += costs[done]
                    done += 1

        plan = [
            ((0, 0), proj_units(1, 0)),
            ((1, 0), proj_units(1, 2)
                     + [(lambda: emit_mc_setup(0), 700),
                        (lambda: emit_mc_setup(1), 700)]),
            ((0, 1), [(lambda b=b: tri_unit(b), 350) for b in (0, 4, 8, 12)]
                     + [(lambda bi=bi: mcb_unit(0, bi), 220)
                        for bi in range(NB)]
                     + pv_units(0, 0) + proj_units(0, 1)[:2]),
            ((1, 1), pv_units(1, 0) + proj_units(0, 1)[2:]
                     + proj_units(0, 3)),
            ((2, 0), proj_units(1, 1)),
            ((3, 0), pv_units(0, 1)),
            ((2, 1), proj_units(1, 3)
                     + [(lambda bi=bi: mcb_unit(1, bi), 220)
                        for bi in range(NB)]),
            ((3, 1), pv_units(1, 1) + pv_units(2, 0) + pv_units(3, 0) + pv_units(2, 1)
                     + pv_units(3, 1)[:6]
                     + [(lambda: pv_unit(3, 1, 14, bj_hi=13), 400),
                        (lambda: pv_unit(3, 1, 15, bj_hi=14), 430)]),
        ]
        for chunk, fillers in plan:
            weave(chunk, fillers)
        pv_unit(3, 1, 14)
        pv_unit(3, 1, 15)


_BUILD_CACHE = {}


def build_nc(alpha, beta, gamma):
    key = (alpha, beta, gamma)
    if key in _BUILD_CACHE:
        return _BUILD_CACHE[key]
    nc = bacc.Bacc("TRN2", target_bir_lowering=False, debug=False,
                   num_devices=NCORES)
    xt = nc.dram_tensor("xt", [C, T], F16, kind="ExternalInput").ap()
    xv = nc.dram_tensor("xv", [T, GC], F16, kind="ExternalInput").ap()
    wt = nc.dram_tensor("wt", [C, 2 * GC], F16, kind="ExternalInput").ap()
    y = nc.dram_tensor("y", [T, GC], F16, kind="ExternalOutput").ap()
    with tile.TileContext(nc) as tc:
        _emit(tc, xt, xv, wt, y, alpha, beta, gamma)
    nc.compile()
    _BUILD_CACHE[key] = nc
    return nc


def make_in_maps(x, w):
    xts = [np.ascontiguousarray(x[b].T).astype(np.float16) for b in range(B)]
    in_maps = []
    for c in range(NCORES):
        b, g = c // HL, c % HL
        wqk = np.concatenate(
            [w[GC * g:GC * (g + 1)], w[C + GC * g:C + GC * (g + 1)]], axis=0)
        in_maps.append({
            "xt": xts[b],
            "xv": np.ascontiguousarray(
                x[b][:, GC * g:GC * (g + 1)]).astype(np.float16),
            "wt": np.ascontiguousarray(wqk.T).astype(np.float16),
        })
    return in_maps


def kernel(x, w_attn, alpha, beta, gamma, n_head, **run_kwargs):
    global LAST_RESULTS
    x = np.asarray(x, dtype=np.float32)
    w = np.asarray(w_attn, dtype=np.float32)
    assert int(n_head) == H and x.shape == (B, T, C)
    nc = build_nc(float(alpha), float(beta), float(gamma))
    res = run_bass_kernel_spmd(nc, make_in_maps(x, w), list(range(NCORES)),
                               **run_kwargs)
    LAST_RESULTS = res
    out = np.empty((B, T, C), dtype=np.float32)
    for c in range(NCORES):
        b, g = c // HL, c % HL
        out[b][:, GC * g:GC * (g + 1)] = res.results[c]["y"].astype(np.float32)
    return out
